# revision 44
# baseline (speedup 1.0000x reference)
"""Contrastive loss (CPC-style) kernel for 8x Trainium2 NeuronCores.

Math: reference computes, for each (step i, time t, sample s), logits over
C=33 targets (1 positive + 32 sampled negatives), then sums
-log_softmax(logits)[0] over all rows.

Reformulation (same as verified baseline, rel-err ~1e-7 vs reference):
  With u = t + i, the 33 gather indices for a row depend only on (s, u).
  Encode them as a multiplicity mask M[s, u, v].  For each (s, u, i):
    se[s,u,i] = sum_v M[s,u,v] * exp(H[s,u,i,v] - 80)
  where H[s,u,i,v] = <pred[s, u-i, :, i], TL[s, v, :]>.  The positive-logit
  sum uses ps[s,f,u] = sum_i pred[s,u-i,f,i]:
    sum_rows H[u,i,u] = sum_{u,f} ps[s,f,u] * TL[s,u,f]
  Rows with u < i contribute exactly log(33); corrected on host.

Device pipeline (per core, 4 samples), transposed H layout:
  1. PE computes Ht[v-block, u] = TL8^T @ pred8 with fp8e4 DoubleRow
     matmuls (2 k-tiles of 128 contracted per instruction, 0.5 cyc/row).
  2. ACT applies exp(x - 80) on [128, 2x512] PSUM -> SBUF bf16.  A sliver
     of groups instead computes exp on DVE via a bf16 Schraudolph bit
     trick (linear-in-bits exp2, then bitcast) to balance ACT/DVE load.
  3. DVE multiplies by the transposed mask mskT[v, u] (bf16, 2x mode).
  4. PE reduces over the 128 v-partitions with a ones-column stationary
     [128, 48] (column c = 12*sl + i), accumulating all (sl, i, vc)
     contributions into one persistent PSUM tile se[48, 512].  These
     reduce-matmuls trail the Gram matmuls by ~9 groups so a late DVE
     tile never stalls PE's in-order stream.
  5. Host takes log(se) in f64 and combines; the positive-logit sum is a
     host-side dot of the same fp8 operands (errors cancel in lse - pos).

Engine budget per core (timeline-sim): ACT 85us, PE 83us, DVE 83us,
Pool 61us, DMA 47us; total ~99us vs ~202us for the dense-bf16 baseline.
"""

import numpy as np
import ml_dtypes

S, T, F = 32, 512, 512
NEG, STEPS = 32, 12
C = NEG + 1
NCORES = 8
SLOC = S // NCORES          # samples per core
KC = F // 128               # contraction chunks
VC = T // 128               # v (target) chunks
PAD = STEPS                 # zero columns in front of each kc row block
ROWB = T + PAD              # padded row block length (524)
SHIFT = 80.0
NC_ROWS = SLOC * STEPS      # 48 rows of the se accumulator

_CACHE: dict = {}


def _split_multi_waits(nc, max_waits: int = 1):
    """This container's walrus accepts at most one sync-wait command per
    instruction; Tile emits more.  Split extras into single-wait NoOps
    preceding the instruction on the same (serial) engine."""
    import concourse.mybir as mybir

    n_split = 0
    for fn in nc.m.functions:
        for bb in fn.blocks:
            new_insts = []
            for inst in bb.instructions:
                si = inst.sync_info
                waits = list(si.on_wait) if si is not None and si.on_wait else []
                if len(waits) > max_waits:
                    head, keep = waits[:-max_waits], waits[-max_waits:]
                    for k, w in enumerate(head):
                        nop = mybir.InstNoOp(
                            name=f"{inst.name}-w{k}",
                            engine=inst.engine,
                            ins=[],
                            outs=[],
                            sync_info=mybir.SyncInfo(on_wait=[w], on_update=[]),
                        )
                        new_insts.append(nop)
                        n_split += 1
                    inst.sync_info = mybir.SyncInfo(
                        on_wait=keep, on_update=list(si.on_update or [])
                    )
                new_insts.append(inst)
            bb.instructions = new_insts
    return n_split


def _build_bass():
    import concourse.bass as bass
    import concourse.mybir as mybir
    from concourse.tile import TileContext

    f32 = mybir.dt.float32
    bf16 = mybir.dt.bfloat16
    fp8 = mybir.dt.float8e4
    i16d = mybir.dt.int16
    ALU = mybir.AluOpType
    AF = mybir.ActivationFunctionType
    DR = mybir.MatmulPerfMode.DoubleRow
    SCH_A = 128.0 / np.log(2.0)                 # bf16 bits per nat
    SCH_B = 16256.0 - SHIFT * SCH_A - 6.42      # bias incl. hw-calibrated c

    nc = bass.Bass()
    predt = nc.dram_tensor("predt", [SLOC, STEPS, F, T], fp8, kind="ExternalInput")
    tlt8 = nc.dram_tensor("tlt8", [SLOC, F, T], fp8, kind="ExternalInput")
    mskT = nc.dram_tensor("mskT", [SLOC, T, T], bf16, kind="ExternalInput")
    out_se = nc.dram_tensor("se", [NC_ROWS, T], f32, kind="ExternalOutput")

    with TileContext(nc) as tc:
        with (
            tc.tile_pool(name="in_pool", bufs=2) as in_pool,
            tc.tile_pool(name="pt_pool", bufs=6) as pt_pool,
            tc.tile_pool(name="eh_pool", bufs=6) as eh_pool,
            tc.tile_pool(name="pr_pool", bufs=12) as pr_pool,
            tc.tile_pool(name="acc_pool", bufs=1) as acc_pool,
            tc.tile_pool(name="psum_h", bufs=3, space="PSUM") as psum_h,
            tc.tile_pool(name="psum_se", bufs=1, space="PSUM") as psum_se,
        ):
            scratch = acc_pool.tile([128, T], bf16)
            nc.gpsimd.memset(scratch[:], 0.0)
            bias_t = acc_pool.tile([128, 1], f32)
            nc.vector.memset(bias_t[:], -SHIFT)
            ones_t = acc_pool.tile([128, 2 * NC_ROWS - 1], bf16)
            nc.vector.memset(ones_t[:], 0.0)
            nc.vector.memset(ones_t[:, NC_ROWS - 1 : NC_ROWS], 1.0)

            se_ps = psum_se.tile([NC_ROWS, T], f32)

            # warm the PE p-state ramp while the first DMAs land; results
            # are discarded (the slot is recycled by the psH pool)
            warm = psum_h.tile([128, 2, T], f32, tag="psH")
            for _ in range(2):
                nc.tensor.matmul(
                    warm[:, 0, :], scratch[:, 0:128], scratch[:],
                    start=True, stop=True, skip_group_check=True,
                )

            # deferred ones-reduce queue for software pipelining of PE
            pending = []
            n_units = SLOC * STEPS * 2  # (sl, i, g) groups
            unit = [0]

            def emit_pending(upto):
                while pending and pending[0][0] <= upto:
                    _, pr_t, g, c = pending.pop(0)
                    first = unit[0] == 0
                    unit[0] += 1
                    last = unit[0] == n_units
                    for vh in range(2):
                        nc.tensor.matmul(
                            se_ps[:],
                            ones_t[:, NC_ROWS - 1 - c : 2 * NC_ROWS - 1 - c],
                            pr_t[:, vh, :],
                            start=(first and vh == 0),
                            stop=(last and vh == 1),
                            skip_group_check=True,
                        )

            def load_sl_inputs(sl):
                tlt8_t = in_pool.tile([128, KC, T], fp8, tag="tlt8")
                nc.sync.dma_start(
                    out=tlt8_t,
                    in_=tlt8[sl].rearrange("(kc p) t -> p kc t", p=128),
                )
                mskT_t = in_pool.tile([128, VC, T], bf16, tag="mskT")
                nc.sync.dma_start(
                    out=mskT_t,
                    in_=mskT[sl].rearrange("(vc p) u -> p vc u", p=128),
                )
                return tlt8_t, mskT_t

            gidx = 0
            pre_pt = {}
            next_in = {}
            for sl in range(SLOC):
                if sl == 0:
                    tlt8_t = in_pool.tile([128, KC, T], fp8, tag="tlt8")
                    nc.sync.dma_start(
                        out=tlt8_t,
                        in_=tlt8[sl].rearrange("(kc p) t -> p kc t", p=128),
                    )
                    # prime the first two pred loads on the fast HWDGE (sync)
                    # queue ahead of the mask so the pipeline starts early
                    for i in (0, 1):
                        pt = pt_pool.tile([128, KC, ROWB], fp8, tag="pt")
                        nc.gpsimd.memset(pt[:, :, 0:PAD], 0.0)
                        nc.sync.dma_start(
                            out=pt[:, :, PAD : PAD + T],
                            in_=predt[sl, i].rearrange("(kc p) t -> p kc t", p=128),
                        )
                        pre_pt[i] = pt
                    mskT_t = in_pool.tile([128, VC, T], bf16, tag="mskT")
                    nc.sync.dma_start(
                        out=mskT_t,
                        in_=mskT[sl].rearrange("(vc p) u -> p vc u", p=128),
                    )
                else:
                    tlt8_t, mskT_t = next_in.pop(sl)
                for i in range(STEPS):
                    # issue the next sample's latents/mask mid-loop so their
                    # transfers queue behind this sample's pred stream
                    if i == 6 and sl + 1 < SLOC:
                        next_in[sl + 1] = load_sl_inputs(sl + 1)
                    pt = pre_pt.pop(i, None)
                    if pt is None:
                        pt = pt_pool.tile([128, KC, ROWB], fp8, tag="pt")
                        nc.gpsimd.memset(pt[:, :, 0:PAD], 0.0)
                        nc.gpsimd.dma_start(
                            out=pt[:, :, PAD : PAD + T],
                            in_=predt[sl, i].rearrange("(kc p) t -> p kc t", p=128),
                        )
                    c = sl * STEPS + i
                    for g in range(2):
                        psH = psum_h.tile([128, 2, T], f32, tag="psH")
                        for vh in range(2):
                            vc = 2 * g + vh
                            for kk in range(2):
                                nc.tensor.matmul(
                                    psH[:, vh, :],
                                    tlt8_t[:, 2 * kk : 2 * kk + 2,
                                           vc * 128 : (vc + 1) * 128],
                                    pt[:, 2 * kk : 2 * kk + 2, PAD - i : PAD - i + T],
                                    start=(kk == 0),
                                    stop=(kk == 1),
                                    perf_mode=DR,
                                )
                        if sl == 0 and i == 0 and g == 0:
                            # split the very first exp so ACT starts as soon
                            # as the first half-group of matmuls lands
                            eh = eh_pool.tile([128, 2, T], bf16, tag="eh")
                            for vh in range(2):
                                nc.scalar.activation(
                                    eh[:, vh, :], psH[:, vh, :], AF.Exp,
                                    bias=bias_t[:],
                                )
                            eh_ap = eh[:]
                        elif (i in (2, 7) and g == 0) or (i == 10 and g == 1) or (
                            i == 4 and g == 1 and sl % 2 == 1):
                            # rebalance: bf16-Schraudolph exp on DVE
                            # (bits = A*(x-80) + 16256 + c, bitcast to bf16)
                            t1 = eh_pool.tile([128, 2, T], f32, tag="sch1")
                            # clamp keeps the biased bit pattern strictly
                            # positive (bits >= +17); -8.0 would go negative
                            # after the -6.42 bias and bitcast to NaN
                            nc.vector.tensor_scalar(
                                t1[:], psH[:], -7.9, SCH_A, op0=ALU.max, op1=ALU.mult
                            )
                            ehs = eh_pool.tile([128, 2, T], i16d, tag="sch2")
                            nc.vector.tensor_scalar(
                                ehs[:], t1[:], SCH_B, None, op0=ALU.add
                            )
                            eh_ap = ehs[:].bitcast(bf16)
                        else:
                            eh = eh_pool.tile([128, 2, T], bf16, tag="eh")
                            nc.scalar.activation(eh[:], psH[:], AF.Exp, bias=bias_t[:])
                            eh_ap = eh[:]
                        pr_t = pr_pool.tile([128, 2, T], bf16, tag="pr")
                        nc.vector.tensor_mul(pr_t[:], eh_ap, mskT_t[:, 2 * g : 2 * g + 2, :])
                        pending.append((gidx, pr_t, g, c))
                        gidx += 1
                        # trail the ones-reduces well behind the Grams for
                        # elasticity; taper near the end (PE idles anyway)
                        delay = min(9, max(3, n_units - gidx + 2))
                        emit_pending(gidx - delay)
            emit_pending(n_units)

            se_sb = acc_pool.tile([NC_ROWS, T], f32)
            nc.vector.tensor_scalar(se_sb[:], se_ps[:], 0.0, None, op0=ALU.add)
            nc.sync.dma_start(out=out_se[:, :], in_=se_sb[:])

    _split_multi_waits(nc)
    return nc


def _get_nc():
    if "nc" not in _CACHE:
        _CACHE["nc"] = _build_bass()
    return _CACHE["nc"]


def _prepare_inputs(true_latent, predictions, neg_indices):
    bf = ml_dtypes.bfloat16
    f8 = ml_dtypes.float8_e4m3
    tl = np.ascontiguousarray(np.asarray(true_latent, np.float32))
    pred = np.asarray(predictions, np.float32)
    ni = np.asarray(neg_indices)

    # predt[s, i, f, t] = pred[s, t, f, i]
    predt = np.ascontiguousarray(pred.transpose(0, 3, 2, 1))
    # tlt[s, f, t] = tl[s, t, f]
    tlt = np.ascontiguousarray(tl.transpose(0, 2, 1))

    # multiplicity mask M[s, u, v]; device uses transposed mskT[s, v, u]
    j = np.arange(NEG * T)
    idx2 = ni + (ni >= (j // NEG)[None, :])
    msk = np.zeros((S, T, T), np.float32)
    rows = np.tile(np.arange(T), NEG)
    for s in range(S):
        np.add.at(msk[s], (rows, idx2[s]), 1.0)
    msk += np.eye(T, dtype=np.float32)[None]
    mskT = np.ascontiguousarray(msk.transpose(0, 2, 1)).astype(bf)

    predt8 = predt.astype(f8)
    tlt8 = tlt.astype(f8)
    # positive-logit sum sum_{s,u,f} ps_q * tlt8 with ps_q built from the
    # same quantized operands the device Gram contracts, so the positive
    # term's quantization error cancels against the lse path in lse - pos
    predt8_f32 = predt8.astype(np.float32)
    ps_q = np.zeros((S, F, T), np.float32)
    for i in range(STEPS):
        ps_q[:, :, i:] += predt8_f32[:, i, :, : T - i]
    pos = np.einsum(
        "sft,sft->", tlt8.astype(np.float32), ps_q, dtype=np.float64
    )

    in_maps = []
    for c in range(NCORES):
        lo, hi = c * SLOC, (c + 1) * SLOC
        in_maps.append(
            {
                "predt": predt8[lo:hi],
                "tlt8": tlt8[lo:hi],
                "mskT": mskT[lo:hi],
            }
        )
    return in_maps, pos


def _combine(results, pos):
    lse = 0.0
    for r in results:
        se = np.asarray(r["se"], np.float64)
        lse += np.log(se).sum()
    n_rows = S * T * STEPS
    n_invalid = S * (STEPS * (STEPS - 1) // 2)
    loss = (lse + SHIFT * n_rows) - pos - n_invalid * np.log(C)
    return np.array([loss], np.float32)


def kernel(true_latent, predictions, neg_indices, **run_kwargs):
    from concourse.bass_utils import run_bass_kernel_spmd

    nc = _get_nc()
    in_maps, pos = _prepare_inputs(true_latent, predictions, neg_indices)
    res = run_bass_kernel_spmd(nc, in_maps, core_ids=list(range(NCORES)), **run_kwargs)
    out = _combine(res.results, pos)
    if run_kwargs:
        _CACHE["last_result"] = res
    return out
